# revision 12
# baseline (speedup 1.0000x reference)
"""Trainium2 Bass kernel for a 4-layer bidirectional-LSTM encoder +
200-step attention decoder (argmax token / dialog-act outputs).

Sharding: data-parallel over batch. B=8 -> 1 batch element per NeuronCore.
All weights replicated; zero collectives; outputs gathered on host.

Per-core layout strategy:
  - hidden dim H=256 lives on SBUF partitions as (128, 2) column pairs
  - all matvecs use weights as lhsT (stationary) so outputs land on
    partitions; biases are injected with K=n "bias matmuls" against an
    identity stationary built from host-packed (n,128) bias matrices
  - LSTM fwd+bwd directions are batched per step; the bwd direction runs
    on reversed time indices (compile-time constants; fully unrolled)
    with carry-masking to reproduce pack/pad semantics
  - only the exp_and_others ACT table set is used (exp + tanh); sigmoid
    is computed as 0.5*tanh(0.5x)+0.5 and ln() via Newton on exp
"""

import sys

sys.path.insert(0, "/opt/trn_rl_repo")

import numpy as np
import ml_dtypes

import concourse.bass as bass
import concourse.mybir as mybir
from concourse import bacc
from concourse.tile import TileContext
from concourse.bass_utils import run_bass_kernel_spmd

F32 = mybir.dt.float32
BF16 = mybir.dt.bfloat16
AF = mybir.ActivationFunctionType
ALU = mybir.AluOpType
AX = mybir.AxisListType

H = 256
NUM_LAYERS = 4
NY = 500
NYP = 512  # padded
NA = 43
B, FEAT = 8, 120
T_FULL, DEC_FULL = 400, 200
NEG_B = 30.0

MM_DT = BF16
MM_NP = ml_dtypes.bfloat16

GATE_MCOLS = [0, 256, 768, 512]  # i, f, o, g weight-column starts


def _bf(x):
    return np.ascontiguousarray(np.asarray(x, np.float32).astype(MM_NP))


def _f32(x):
    return np.ascontiguousarray(np.asarray(x, np.float32))


# ----------------------------------------------------------------------------
# host-side packing
# ----------------------------------------------------------------------------

def pack_weights(lstm_params, params):
    shared = {}
    for l in range(NUM_LAYERS):
        for d in range(2):
            p = lstm_params[2 * l + d]
            shared[f"wih_{l}_{d}"] = _bf(p["Wih"])
            shared[f"whh_{l}_{d}"] = _bf(p["Whh"])
            shared[f"b_{l}_{d}"] = _f32(np.asarray(p["b"], np.float32)[None, :])
    pr = {k: np.asarray(v, np.float32) for k, v in params.items()}

    shared["atts_w"] = _bf(pr["atts_w"])
    shared["atth_w"] = _bf(pr["atth_w"])
    shared["atth_b"] = _f32(pr["atth_b"][None])
    shared["attw_w"] = _bf(pr["attw_w"])
    shared["sy_w"] = _bf(pr["sy_w"])
    shared["gy_w"] = _bf(pr["gy_w"])
    yy = np.zeros((H, NYP), np.float32)
    yy[:, :NY] = pr["yy_w"]
    shared["yy_w"] = _bf(yy)
    ys = np.zeros((NYP, 4 * H), np.float32)
    ys[:NY] = pr["ys_w"]
    shared["ys_w"] = _bf(ys)
    shared["ss_w"] = _bf(pr["ss_w"])
    shared["gs_w"] = _bf(pr["gs_w"])
    shared["sn_w"] = _bf(pr["sn_w"])
    shared["nn_w"] = _bf(pr["nn_w"])
    shared["wn_w"] = _bf(pr["wn_w"])
    shared["dan_w"] = _bf(pr["dan_w"])
    shared["nda_w"] = _bf(pr["nda_w"])
    shared["nda_b"] = _f32(pr["nda_b"][None])

    shared["bias2"] = _f32(pr["gy_b"].reshape(2, 128))
    b10 = np.zeros((10, 128), np.float32)
    zb = (pr["gs_b"] + pr["ys_b"]).reshape(8, 128)
    b10[0:8] = zb[[0, 1, 2, 3, 6, 7, 4, 5]]
    b10[8:10] = (pr["nn_b"] + pr["dan_b"]).reshape(2, 128)
    shared["bias10"] = _f32(b10)
    yyb = np.full((4, 128), -1.0e9, np.float32)
    yyb.reshape(-1)[:NY] = pr["yy_b"]
    shared["yyb"] = _f32(yyb)
    shared["wnb"] = _f32(pr["wn_b"].reshape(2, 128))

    ident = np.eye(128, dtype=np.float32)
    shared["ident"] = ident
    shared["identb"] = _bf(ident)
    shared["iota_y"] = _f32((1.0 + np.arange(NYP, dtype=np.float32))[None])
    shared["iota_a"] = _f32((1.0 + np.arange(NA, dtype=np.float32))[None])
    return shared


def pack_percore(shared, data, length, T):
    data = np.asarray(data, np.float32)
    length = np.asarray(length, np.int32)
    shared = dict(shared)
    shared["onesrow"] = _f32(np.ones((1, T), np.float32))
    per_core = []
    for b in range(data.shape[0]):
        m = dict(shared)
        m["xt0"] = _bf(data[b].T)
        mask = (np.arange(T) < int(length[b])).astype(np.float32)
        m["maskb"] = _f32(np.broadcast_to(mask[None, :], (128, T)))
        m["maskneg"] = _f32(((mask - 1.0) * NEG_B)[None, :])
        per_core.append(m)
    return per_core


# ----------------------------------------------------------------------------
# bass program
# ----------------------------------------------------------------------------

def _sig(nc, ap, out=None):
    """sigmoid via tanh: 0.5*tanh(0.5x)+0.5 (stays in exp_and_others set)"""
    dst = ap if out is None else out
    nc.scalar.activation(dst, ap, AF.Tanh, scale=0.5)
    nc.vector.tensor_scalar(dst, dst, 0.5, 0.5, ALU.mult, ALU.add)


def build_nc(T=T_FULL, DEC=DEC_FULL, num_devices=8):
    nc = bacc.Bacc("TRN2", target_bir_lowering=False, debug=False,
                   num_devices=num_devices)

    def inp(name, shape, dt=MM_DT):
        return nc.declare_dram_parameter(name, list(shape), dt, isOutput=False)

    d_wih, d_whh, d_b = [], [], []
    for l in range(NUM_LAYERS):
        din = FEAT if l == 0 else 2 * H
        for d in range(2):
            d_wih.append(inp(f"wih_{l}_{d}", (din, 4 * H)))
            d_whh.append(inp(f"whh_{l}_{d}", (H, 4 * H)))
            d_b.append(inp(f"b_{l}_{d}", (1, 4 * H), F32))
    d_xt0 = inp("xt0", (FEAT, T))
    d_maskb = inp("maskb", (128, T), F32)
    d_maskneg = inp("maskneg", (1, T), F32)
    d_atts = inp("atts_w", (H, 2 * H))
    d_atth = inp("atth_w", (2 * H, 2 * H))
    d_atthb = inp("atth_b", (1, 2 * H), F32)
    d_attw = inp("attw_w", (2 * H, 1))
    d_sy = inp("sy_w", (H, H))
    d_gy = inp("gy_w", (2 * H, H))
    d_yy = inp("yy_w", (H, NYP))
    d_ys = inp("ys_w", (NYP, 4 * H))
    d_ss = inp("ss_w", (H, 4 * H))
    d_gs = inp("gs_w", (2 * H, 4 * H))
    d_sn = inp("sn_w", (H, H))
    d_nn = inp("nn_w", (H, H))
    d_wn = inp("wn_w", (H, H))
    d_dan = inp("dan_w", (NA, H))
    d_nda = inp("nda_w", (H, NA))
    d_ndab = inp("nda_b", (1, NA), F32)
    d_bias2 = inp("bias2", (2, 128), F32)
    d_bias10 = inp("bias10", (10, 128), F32)
    d_yyb = inp("yyb", (4, 128), F32)
    d_wnb = inp("wnb", (2, 128), F32)
    d_ident = inp("ident", (128, 128), F32)
    d_identb = inp("identb", (128, 128))
    d_iotay = inp("iota_y", (1, NYP), F32)
    d_iotaa = inp("iota_a", (1, NA), F32)
    d_ones = inp("onesrow", (1, T), F32)

    d_yout = nc.declare_dram_parameter("yout", [1, DEC], F32, isOutput=True)
    d_Yout = nc.declare_dram_parameter("Yout", [1, DEC], F32, isOutput=True)

    nch = (T + 127) // 128  # time chunks for transposes

    with TileContext(nc) as tc:
        with tc.tile_pool(name="const", bufs=1) as cp, \
             tc.tile_pool(name="xbuf", bufs=2) as xp, \
             tc.tile_pool(name="state", bufs=1) as st, \
             tc.tile_pool(name="work", bufs=4) as wk:

            msk = cp.tile([128, T], F32)
            nc.sync.dma_start(out=msk[:], in_=d_maskb[:])
            mskneg = cp.tile([1, T], F32)
            nc.sync.dma_start(out=mskneg[:], in_=d_maskneg[:])
            ones = cp.tile([1, T], F32)
            nc.sync.dma_start(out=ones[:], in_=d_ones[:])
            ident = cp.tile([128, 128], F32)
            nc.sync.dma_start(out=ident[:], in_=d_ident[:])
            identb = cp.tile([128, 128], MM_DT)
            nc.sync.dma_start(out=identb[:], in_=d_identb[:])
            iotay = cp.tile([1, NYP], F32)
            nc.sync.dma_start(out=iotay[:], in_=d_iotay[:])
            iotaa = cp.tile([1, NA], F32)
            nc.sync.dma_start(out=iotaa[:], in_=d_iotaa[:])

            # ================= ENCODER =================
            x0 = xp.tile([128, 1, T], MM_DT, tag="x0")
            nc.sync.dma_start(out=x0[0:FEAT, 0, :], in_=d_xt0[:])
            xcur = x0
            with tc.tile_pool(name="encw", bufs=2) as wp, \
                 tc.tile_pool(name="encbig", bufs=1) as bigp, \
                 tc.tile_pool(name="eps", bufs=2, space="PSUM") as eps, \
                 tc.tile_pool(name="epre", bufs=2, space="PSUM") as epre:
                for l in range(NUM_LAYERS):
                    din = FEAT if l == 0 else 2 * H
                    nK = (din + 127) // 128
                    kp = [min(128, din - 128 * k) for k in range(nK)]

                    wih = [wp.tile([128, nK, 4 * H], MM_DT, tag=f"wih{d}", name=f"wih{l}{d}") for d in range(2)]
                    whh = [wp.tile([128, 2, 4 * H], MM_DT, tag=f"whh{d}", name=f"whh{l}{d}") for d in range(2)]
                    bb = [wp.tile([1, 4 * H], F32, tag=f"bb{d}", name=f"bb{l}{d}") for d in range(2)]
                    for d in range(2):
                        src = d_wih[2 * l + d]
                        for k in range(nK):
                            nc.sync.dma_start(
                                out=wih[d][0:kp[k], k, :],
                                in_=src[128 * k : 128 * k + kp[k], :])
                        nc.sync.dma_start(
                            out=whh[d][:],
                            in_=d_whh[2 * l + d].rearrange("(k p) m -> p k m", p=128))
                        nc.sync.dma_start(out=bb[d][:], in_=d_b[2 * l + d][:])

                    # xz: (128, 16, T); cols [fwd i,i,f,f,o,o,g,g | bwd same]
                    xz = bigp.tile([128, 16, T], F32, tag="xz")
                    for d in range(2):
                        for gi, g0 in enumerate(GATE_MCOLS):
                            for jj in range(2):
                                col = d * 8 + gi * 2 + jj
                                mc = slice(g0 + jj * 128, g0 + (jj + 1) * 128)
                                acc = epre.tile([128, T], F32, tag="pre")
                                nc.tensor.matmul(acc[:], bb[d][0:1, mc], ones[:],
                                                 start=True, stop=False)
                                for k in range(nK):
                                    nc.tensor.matmul(
                                        acc[:],
                                        wih[d][0:kp[k], k, mc],
                                        xcur[0:kp[k], k, :],
                                        start=False, stop=(k == nK - 1))
                                nc.vector.tensor_copy(xz[:, col, :], acc[:])

                    xnext = xp.tile([128, 4, T], MM_DT, tag="xn")
                    h = st.tile([128, 2, 2], F32, tag="h")
                    hbf = st.tile([128, 2, 2], MM_DT, tag="hbf")
                    c = st.tile([128, 2, 2], F32, tag="c")
                    nc.vector.memset(h[:], 0.0)
                    nc.vector.memset(hbf[:], 0.0)
                    nc.vector.memset(c[:], 0.0)

                    for t in range(T):
                        tf, tb = t, T - 1 - t
                        z = eps.tile([128, 2, 8], F32, tag="z")
                        for d in range(2):
                            for gi, g0 in enumerate(GATE_MCOLS):
                                for jj in range(2):
                                    for k in range(2):
                                        nc.tensor.matmul(
                                            z[:, d, gi * 2 + jj : gi * 2 + jj + 1],
                                            whh[d][:, k, g0 + jj * 128 : g0 + (jj + 1) * 128],
                                            hbf[:, d, k : k + 1],
                                            start=(k == 0), stop=(k == 1))
                        zz = wk.tile([128, 2, 8], F32, tag="zz")
                        nc.vector.tensor_tensor(zz[:, 0, :], z[:, 0, :], xz[:, 0:8, tf], ALU.add)
                        nc.vector.tensor_tensor(zz[:, 1, :], z[:, 1, :], xz[:, 8:16, tb], ALU.add)
                        _sig(nc, zz[:, :, 0:6])
                        nc.scalar.activation(zz[:, :, 6:8], zz[:, :, 6:8], AF.Tanh)
                        tmp = wk.tile([128, 2, 2], F32, tag="tmp")
                        nc.vector.tensor_tensor(tmp[:], zz[:, :, 0:2], zz[:, :, 6:8], ALU.mult)
                        nc.vector.tensor_tensor(c[:], zz[:, :, 2:4], c[:], ALU.mult)
                        nc.vector.tensor_tensor(c[:], c[:], tmp[:], ALU.add)
                        tct = wk.tile([128, 2, 2], F32, tag="tct")
                        nc.scalar.activation(tct[:], c[:], AF.Tanh)
                        nc.vector.tensor_tensor(h[:], zz[:, :, 4:6], tct[:], ALU.mult)
                        nc.vector.tensor_scalar(xnext[:, 0:2, tf], h[:, 0, :],
                                                msk[:, tf : tf + 1], None, ALU.mult)
                        nc.vector.tensor_scalar(h[:, 1, :], h[:, 1, :],
                                                msk[:, tb : tb + 1], None, ALU.mult)
                        nc.vector.tensor_scalar(c[:, 1, :], c[:, 1, :],
                                                msk[:, tb : tb + 1], None, ALU.mult)
                        nc.vector.tensor_copy(xnext[:, 2:4, tb], h[:, 1, :])
                        nc.vector.tensor_copy(hbf[:], h[:])
                    xcur = xnext

            hpack = xcur  # (128, 4, T) f32, k-partition layout, masked

            # ================= DECODER =================
            with tc.tile_pool(name="decw", bufs=1) as dw, \
                 tc.tile_pool(name="decbig", bufs=1) as db, \
                 tc.tile_pool(name="dlin", bufs=1, space="PSUM") as dlin, \
                 tc.tile_pool(name="dscr", bufs=4, space="PSUM") as dscr:

                hp_bf = hpack  # already bf16
                watth = dw.tile([128, 4, 2 * H], MM_DT, tag="watth")
                nc.sync.dma_start(out=watth[:],
                                  in_=d_atth.rearrange("(k p) m -> p k m", p=128))
                batth = dw.tile([1, 2 * H], F32, tag="batth")
                nc.sync.dma_start(out=batth[:], in_=d_atthb[:])

                ah = db.tile([128, 4, T], F32, tag="ah")
                for m in range(4):
                    mc = slice(m * 128, (m + 1) * 128)
                    acc = dscr.tile([128, T], F32, tag="scr")
                    nc.tensor.matmul(acc[:], batth[0:1, mc], ones[:],
                                     start=True, stop=False)
                    for k in range(4):
                        nc.tensor.matmul(acc[:], watth[:, k, mc], hp_bf[:, k, :],
                                         start=False, stop=(k == 3))
                    nc.vector.tensor_copy(ah[:, m, :], acc[:])

                hp_t = db.tile([128, nch, 2 * H], MM_DT, tag="hpt")
                for cch in range(nch):
                    cw = min(128, T - cch * 128)
                    for j in range(4):
                        tp = dscr.tile([128, 128], MM_DT, tag="scr")
                        nc.tensor.matmul(tp[0:cw, 0:128],
                                         hp_bf[:, j, cch * 128 : cch * 128 + cw],
                                         identb[:], is_transpose=True,
                                         start=True, stop=True)
                        nc.vector.tensor_copy(
                            hp_t[0:cw, cch, j * 128 : (j + 1) * 128], tp[0:cw, 0:128])

                def wtile(name, dram, kdim, mdim):
                    nK2 = (kdim + 127) // 128
                    tl = dw.tile([128, nK2, mdim], MM_DT, tag=name)
                    if kdim % 128 == 0:
                        nc.sync.dma_start(out=tl[:],
                                          in_=dram.rearrange("(k p) m -> p k m", p=128))
                    else:
                        nc.sync.dma_start(out=tl[0:kdim, 0, :], in_=dram[:])
                    return tl

                w_atts = wtile("w_atts", d_atts, H, 2 * H)
                w_sy = wtile("w_sy", d_sy, H, H)
                w_ss = wtile("w_ss", d_ss, H, 4 * H)
                w_sn = wtile("w_sn", d_sn, H, H)
                w_gy = wtile("w_gy", d_gy, 2 * H, H)
                w_gs = wtile("w_gs", d_gs, 2 * H, 4 * H)
                w_yy = wtile("w_yy", d_yy, H, NYP)
                w_ys = wtile("w_ys", d_ys, NYP, 4 * H)
                w_nn = wtile("w_nn", d_nn, H, H)
                w_wn = wtile("w_wn", d_wn, H, H)
                w_dan = wtile("w_dan", d_dan, NA, H)
                w_nda = wtile("w_nda", d_nda, H, NA)
                w_attw = dw.tile([128, 4, 1], MM_DT, tag="w_attw")
                nc.sync.dma_start(out=w_attw[:],
                                  in_=d_attw.rearrange("(k p) m -> p k m", p=128))

                bias2 = dw.tile([2, 128], F32, tag="bias2")
                nc.sync.dma_start(out=bias2[:], in_=d_bias2[:])
                bias10 = dw.tile([10, 128], F32, tag="bias10")
                nc.sync.dma_start(out=bias10[:], in_=d_bias10[:])
                yyb = dw.tile([4, 128], F32, tag="yyb")
                nc.sync.dma_start(out=yyb[:], in_=d_yyb[:])
                wnb = dw.tile([2, 128], F32, tag="wnb")
                nc.sync.dma_start(out=wnb[:], in_=d_wnb[:])
                ndab = dw.tile([1, NA], F32, tag="ndab")
                nc.sync.dma_start(out=ndab[:], in_=d_ndab[:])
                onec = dw.tile([1, 1], F32, tag="onec")
                nc.vector.memset(onec[:], 1.0)
                lnS0 = dw.tile([1, 1], F32, tag="lnS0")
                nc.vector.memset(lnS0[:], 3.7612)

                sbf = st.tile([128, 2], MM_DT, tag="sbf")
                nbf = st.tile([128, 2], MM_DT, tag="nbf")
                c2 = st.tile([128, 2], F32, tag="c2")
                nc.vector.memset(sbf[:], 0.0)
                nc.vector.memset(nbf[:], 0.0)
                nc.vector.memset(c2[:], 0.0)

                youts = st.tile([1, DEC], F32, tag="youts")
                Youts = st.tile([1, DEC], F32, tag="Youts")

                for it in range(DEC):
                    # q_ps (4) | yp_ps (2) | zn_ps: [z(8) npre(2)]
                    q_ps = dlin.tile([128, 4], F32, tag="qps")
                    for m in range(4):
                        for k in range(2):
                            nc.tensor.matmul(q_ps[:, m : m + 1],
                                             w_atts[:, k, m * 128 : (m + 1) * 128],
                                             sbf[:, k : k + 1],
                                             start=(k == 0), stop=(k == 1))
                    yp_ps = dlin.tile([128, 2], F32, tag="ypps")
                    nc.tensor.matmul(yp_ps[:], bias2[:], ident[0:2, 0:2],
                                     start=True, stop=False)
                    zn_ps = dlin.tile([128, 10], F32, tag="znps")
                    nc.tensor.matmul(zn_ps[:], bias10[:], ident[0:10, 0:10],
                                     start=True, stop=False)
                    for k in range(2):
                        rh = sbf[:, k : k + 1]
                        for m in range(2):
                            nc.tensor.matmul(yp_ps[:, m : m + 1],
                                             w_sy[:, k, m * 128 : (m + 1) * 128], rh,
                                             start=False, stop=False)
                        for m in range(8):
                            zc = GATE_MCOLS[m // 2] + (m % 2) * 128
                            nc.tensor.matmul(zn_ps[:, m : m + 1],
                                             w_ss[:, k, zc : zc + 128], rh,
                                             start=False, stop=False)
                        for m in range(2):
                            nc.tensor.matmul(zn_ps[:, 8 + m : 9 + m],
                                             w_sn[:, k, m * 128 : (m + 1) * 128], rh,
                                             start=False, stop=False)
                    qsb = wk.tile([128, 4], F32, tag="qsb")
                    nc.vector.tensor_copy(qsb[:], q_ps[:])

                    th = wk.tile([128, 4, T], MM_DT, tag="th")
                    for j in range(4):
                        nc.scalar.activation(th[:, j, :], ah[:, j, :], AF.Tanh,
                                             bias=qsb[:, j : j + 1])
                    e_ps = dscr.tile([1, T], F32, tag="scr")
                    nc.tensor.matmul(e_ps[:], onec[:], mskneg[:], start=True, stop=False)
                    for j in range(4):
                        nc.tensor.matmul(e_ps[:], w_attw[:, j, :], th[:, j, :],
                                         start=False, stop=(j == 3))
                    en = wk.tile([1, T], F32, tag="en")
                    nc.scalar.activation(en[:], e_ps[:], AF.Exp)
                    S = wk.tile([1, 1], F32, tag="S")
                    nc.vector.tensor_reduce(S[:], en[:], AX.X, ALU.add)
                    rS = wk.tile([1, 1], F32, tag="rS")
                    nc.vector.reciprocal(rS[:], S[:])
                    enT = dscr.tile([128, nch], F32, tag="scr")
                    for cch in range(nch):
                        cw = min(128, T - cch * 128)
                        nc.tensor.matmul(enT[0:cw, cch : cch + 1],
                                         en[0:1, cch * 128 : cch * 128 + cw],
                                         ident[0:1, 0:1], is_transpose=True,
                                         start=True, stop=True)
                    enb = wk.tile([128, nch], MM_DT, tag="enb")
                    for cch in range(nch):
                        cw = min(128, T - cch * 128)
                        nc.vector.tensor_copy(enb[0:cw, cch : cch + 1],
                                              enT[0:cw, cch : cch + 1])
                    g_ps = dscr.tile([1, 2 * H], F32, tag="scr")
                    for cch in range(nch):
                        cw = min(128, T - cch * 128)
                        nc.tensor.matmul(g_ps[:], enb[0:cw, cch : cch + 1],
                                         hp_t[0:cw, cch, :],
                                         start=(cch == 0), stop=(cch == nch - 1))
                    gsc = wk.tile([1, 2 * H], F32, tag="gsc")
                    nc.vector.tensor_scalar(gsc[:], g_ps[:], rS[:], None, ALU.mult)
                    gT = dscr.tile([128, 4], F32, tag="scr")
                    for j in range(4):
                        nc.tensor.matmul(gT[:, j : j + 1],
                                         gsc[0:1, j * 128 : (j + 1) * 128],
                                         ident[0:1, 0:1], is_transpose=True,
                                         start=True, stop=True)
                    gbf = wk.tile([128, 4], MM_DT, tag="gbf")
                    nc.vector.tensor_copy(gbf[:], gT[:])

                    for k in range(4):
                        rh = gbf[:, k : k + 1]
                        for m in range(2):
                            nc.tensor.matmul(yp_ps[:, m : m + 1],
                                             w_gy[:, k, m * 128 : (m + 1) * 128], rh,
                                             start=False, stop=(k == 3 and m == 1))
                        for m in range(8):
                            zc = GATE_MCOLS[m // 2] + (m % 2) * 128
                            nc.tensor.matmul(zn_ps[:, m : m + 1],
                                             w_gs[:, k, zc : zc + 128], rh,
                                             start=False, stop=False)

                    yhat = wk.tile([128, 2], MM_DT, tag="yhat")
                    nc.scalar.activation(yhat[:], yp_ps[:], AF.Tanh)
                    y_ps = dscr.tile([128, 4], F32, tag="scr")
                    nc.tensor.matmul(y_ps[:], yyb[:], ident[0:4, 0:4],
                                     start=True, stop=False)
                    for k in range(2):
                        for m in range(4):
                            nc.tensor.matmul(y_ps[:, m : m + 1],
                                             w_yy[:, k, m * 128 : (m + 1) * 128],
                                             yhat[:, k : k + 1],
                                             start=False, stop=(k == 1 and m == 3))
                    ylb = wk.tile([128, 4], MM_DT, tag="ylb")
                    nc.vector.tensor_copy(ylb[:], y_ps[:])
                    ysb = wk.tile([128, 4], F32, tag="ysb")
                    nc.vector.tensor_copy(ysb[:], y_ps[:])
                    yfree = dscr.tile([1, NYP], F32, tag="scr")
                    for j in range(4):
                        nc.tensor.matmul(yfree[0:1, j * 128 : (j + 1) * 128],
                                         ysb[:, j : j + 1], ident[:, :],
                                         is_transpose=True, start=True, stop=True)
                    ym = wk.tile([1, 1], F32, tag="ym")
                    nc.vector.tensor_reduce(ym[:], yfree[:], AX.X, ALU.max)
                    yam = wk.tile([1, NYP], F32, tag="yam")
                    nc.vector.scalar_tensor_tensor(yam[:], yfree[:], ym[:], iotay[:],
                                                   ALU.is_ge, ALU.mult)
                    yi = wk.tile([1, 1], F32, tag="yi")
                    nc.vector.tensor_reduce(yi[:], yam[:], AX.X, ALU.max)
                    nc.vector.tensor_scalar(youts[:, it : it + 1], yi[:],
                                            -1.0, None, ALU.add)

                    da_ps = dscr.tile([1, NA], F32, tag="scr")
                    nc.tensor.matmul(da_ps[:], onec[:], ndab[:], start=True, stop=False)
                    for k in range(2):
                        nc.tensor.matmul(da_ps[:], nbf[:, k : k + 1],
                                         w_nda[:, k, :], start=False, stop=(k == 1))
                    dm = wk.tile([1, 1], F32, tag="dm")
                    nc.vector.tensor_reduce(dm[:], da_ps[:], AX.X, ALU.max)
                    dam = wk.tile([1, NA], F32, tag="dam")
                    nc.vector.scalar_tensor_tensor(dam[:], da_ps[:], dm[:], iotaa[:],
                                                   ALU.is_ge, ALU.mult)
                    di = wk.tile([1, 1], F32, tag="di")
                    nc.vector.tensor_reduce(di[:], dam[:], AX.X, ALU.max)
                    nc.vector.tensor_scalar(Youts[:, it : it + 1], di[:],
                                            -1.0, None, ALU.add)

                    eda = wk.tile([1, NA], F32, tag="eda")
                    nc.scalar.activation(eda[:], da_ps[:], AF.Exp)
                    Sd = wk.tile([1, 1], F32, tag="Sd")
                    nc.vector.tensor_reduce(Sd[:], eda[:], AX.X, ALU.add)
                    lse = wk.tile([1, 1], F32, tag="lse")
                    nc.vector.tensor_copy(lse[:], lnS0[:])
                    for _ in range(3):
                        u = wk.tile([1, 1], F32, tag="u")
                        nc.scalar.activation(u[:], lse[:], AF.Exp, scale=-1.0)
                        nc.vector.scalar_tensor_tensor(lse[:], u[:], Sd[:], lse[:],
                                                       ALU.mult, ALU.add)
                        nc.vector.tensor_scalar(lse[:], lse[:], -1.0, None, ALU.add)
                    neglse = wk.tile([1, 1], F32, tag="neglse")
                    nc.vector.tensor_scalar(neglse[:], lse[:], -1.0, None, ALU.mult)
                    logp = wk.tile([1, NA], MM_DT, tag="logp")
                    nc.vector.tensor_scalar(logp[:], da_ps[:], neglse[:], None, ALU.add)
                    lpT = dscr.tile([NA, 1], MM_DT, tag="scr")
                    nc.tensor.matmul(lpT[:], logp[0:1, :], identb[0:1, 0:1],
                                     is_transpose=True, start=True, stop=True)
                    lpb = wk.tile([NA, 1], MM_DT, tag="lpb")
                    nc.vector.tensor_copy(lpb[:], lpT[:])

                    for k in range(2):
                        for m in range(2):
                            nc.tensor.matmul(zn_ps[:, 8 + m : 9 + m],
                                             w_nn[:, k, m * 128 : (m + 1) * 128],
                                             nbf[:, k : k + 1], start=False, stop=False)
                    for m in range(2):
                        nc.tensor.matmul(zn_ps[:, 8 + m : 9 + m],
                                         w_dan[0:NA, 0, m * 128 : (m + 1) * 128],
                                         lpb[:], start=False, stop=False)
                    for k in range(4):
                        for m in range(8):
                            zc = GATE_MCOLS[m // 2] + (m % 2) * 128
                            nc.tensor.matmul(zn_ps[:, m : m + 1],
                                             w_ys[:, k, zc : zc + 128],
                                             ylb[:, k : k + 1],
                                             start=False, stop=(k == 3 and m == 7))

                    nhat = wk.tile([128, 2], MM_DT, tag="nhat")
                    nc.scalar.activation(nhat[:], zn_ps[:, 8:10], AF.Tanh)
                    n_ps = dscr.tile([128, 2], F32, tag="scr")
                    nc.tensor.matmul(n_ps[:], wnb[:], ident[0:2, 0:2],
                                     start=True, stop=False)
                    for k in range(2):
                        for m in range(2):
                            nc.tensor.matmul(n_ps[:, m : m + 1],
                                             w_wn[:, k, m * 128 : (m + 1) * 128],
                                             nhat[:, k : k + 1],
                                             start=False, stop=(k == 1 and m == 1))
                    nc.vector.tensor_copy(nbf[:], n_ps[:])

                    zz2 = wk.tile([128, 8], F32, tag="zz2")
                    _sig(nc, zn_ps[:, 0:6], out=zz2[:, 0:6])
                    nc.scalar.activation(zz2[:, 6:8], zn_ps[:, 6:8], AF.Tanh)
                    tmp2 = wk.tile([128, 2], F32, tag="tmp2")
                    nc.vector.tensor_tensor(tmp2[:], zz2[:, 0:2], zz2[:, 6:8], ALU.mult)
                    nc.vector.tensor_tensor(c2[:], zz2[:, 2:4], c2[:], ALU.mult)
                    nc.vector.tensor_tensor(c2[:], c2[:], tmp2[:], ALU.add)
                    tc2 = wk.tile([128, 2], F32, tag="tc2")
                    nc.scalar.activation(tc2[:], c2[:], AF.Tanh)
                    nc.vector.tensor_tensor(sbf[:], zz2[:, 4:6], tc2[:], ALU.mult)

                nc.sync.dma_start(out=d_yout[:], in_=youts[:])
                nc.sync.dma_start(out=d_Yout[:], in_=Youts[:])

    nc.compile()
    return nc


_NC = None


def kernel(data, length, lstm_params, params):
    global _NC
    shared = pack_weights(lstm_params, params)
    per_core = pack_percore(shared, data, length, T_FULL)
    if _NC is None:
        _NC = build_nc()
    res = run_bass_kernel_spmd(_NC, per_core, list(range(B)))
    yout = np.stack([_f32(res.results[b]["yout"])[0] for b in range(B)])
    Yout = np.stack([_f32(res.results[b]["Yout"])[0] for b in range(B)])
    return (yout, Yout)


# revision 13
# speedup vs baseline: 1.1912x; 1.1912x over previous
"""Trainium2 Bass kernel for a 4-layer bidirectional-LSTM encoder +
200-step attention decoder (argmax token / dialog-act outputs).

Sharding: data-parallel over batch. B=8 -> 1 batch element per NeuronCore.
All weights replicated; zero collectives; outputs gathered on host.

Per-core layout strategy:
  - hidden dim H=256 lives on SBUF partitions as (128, 2) column pairs
  - all matvecs use weights as lhsT (stationary) so outputs land on
    partitions; biases are injected with K=n "bias matmuls" against an
    identity stationary built from host-packed (n,128) bias matrices
  - LSTM fwd+bwd directions are batched per step; the bwd direction runs
    on reversed time indices (compile-time constants; fully unrolled)
    with carry-masking to reproduce pack/pad semantics
  - only the exp_and_others ACT table set is used (exp + tanh); sigmoid
    is computed as 0.5*tanh(0.5x)+0.5 and ln() via Newton on exp
"""

import sys

sys.path.insert(0, "/opt/trn_rl_repo")

import numpy as np
import ml_dtypes

import concourse.bass as bass
import concourse.mybir as mybir
from concourse import bacc
from concourse.tile import TileContext
from concourse.bass_utils import run_bass_kernel_spmd

F32 = mybir.dt.float32
BF16 = mybir.dt.bfloat16
AF = mybir.ActivationFunctionType
ALU = mybir.AluOpType
AX = mybir.AxisListType

H = 256
NUM_LAYERS = 4
NY = 500
NYP = 512  # padded
NA = 43
B, FEAT = 8, 120
T_FULL, DEC_FULL = 400, 200
NEG_B = 30.0

MM_DT = BF16
MM_NP = ml_dtypes.bfloat16

GATE_MCOLS = [0, 256, 768, 512]  # i, f, o, g weight-column starts


def _bf(x):
    return np.ascontiguousarray(np.asarray(x, np.float32).astype(MM_NP))


def _f32(x):
    return np.ascontiguousarray(np.asarray(x, np.float32))


# ----------------------------------------------------------------------------
# host-side packing
# ----------------------------------------------------------------------------

def pack_weights(lstm_params, params):
    shared = {}
    for l in range(NUM_LAYERS):
        for d in range(2):
            p = lstm_params[2 * l + d]
            shared[f"wih_{l}_{d}"] = _bf(p["Wih"])
            shared[f"whh_{l}_{d}"] = _bf(p["Whh"])
            shared[f"b_{l}_{d}"] = _f32(np.asarray(p["b"], np.float32)[None, :])
    pr = {k: np.asarray(v, np.float32) for k, v in params.items()}

    shared["atts_w"] = _bf(pr["atts_w"])
    shared["atth_w"] = _bf(pr["atth_w"])
    shared["atth_b"] = _f32(pr["atth_b"][None])
    shared["attw_w"] = _bf(pr["attw_w"])
    shared["sy_w"] = _bf(pr["sy_w"])
    shared["gy_w"] = _bf(pr["gy_w"])
    yy = np.zeros((H, NYP), np.float32)
    yy[:, :NY] = pr["yy_w"]
    shared["yy_w"] = _bf(yy)
    ys = np.zeros((NYP, 4 * H), np.float32)
    ys[:NY] = pr["ys_w"]
    shared["ys_w"] = _bf(ys)
    shared["ss_w"] = _bf(pr["ss_w"])
    shared["gs_w"] = _bf(pr["gs_w"])
    shared["sn_w"] = _bf(pr["sn_w"])
    shared["nn_w"] = _bf(pr["nn_w"])
    shared["wn_w"] = _bf(pr["wn_w"])
    shared["dan_w"] = _bf(pr["dan_w"])
    shared["nda_w"] = _bf(pr["nda_w"])
    shared["nda_b"] = _f32(pr["nda_b"][None])

    shared["bias2"] = _f32(pr["gy_b"].reshape(2, 128))
    b10 = np.zeros((10, 128), np.float32)
    zb = (pr["gs_b"] + pr["ys_b"]).reshape(8, 128)
    b10[0:8] = zb[[0, 1, 2, 3, 6, 7, 4, 5]]
    b10[8:10] = (pr["nn_b"] + pr["dan_b"]).reshape(2, 128)
    shared["bias10"] = _f32(b10)
    yyb = np.full((4, 128), -1.0e9, np.float32)
    yyb.reshape(-1)[:NY] = pr["yy_b"]
    shared["yyb"] = _f32(yyb)
    shared["wnb"] = _f32(pr["wn_b"].reshape(2, 128))

    ident = np.eye(128, dtype=np.float32)
    shared["ident"] = ident
    shared["identb"] = _bf(ident)
    shared["iota_y"] = _f32((1.0 + np.arange(NYP, dtype=np.float32))[None])
    shared["iota_a"] = _f32((1.0 + np.arange(NA, dtype=np.float32))[None])
    return shared


def pack_percore(shared, data, length, T):
    data = np.asarray(data, np.float32)
    length = np.asarray(length, np.int32)
    shared = dict(shared)
    shared["onesrow"] = _f32(np.ones((1, T), np.float32))
    per_core = []
    for b in range(data.shape[0]):
        m = dict(shared)
        m["xt0"] = _bf(data[b].T)
        mask = (np.arange(T) < int(length[b])).astype(np.float32)
        m["maskb"] = _f32(np.broadcast_to(mask[None, :], (128, T)))
        m["maskneg"] = _f32(((mask - 1.0) * NEG_B)[None, :])
        per_core.append(m)
    return per_core


# ----------------------------------------------------------------------------
# bass program
# ----------------------------------------------------------------------------

def _sig(nc, ap, out=None):
    """sigmoid via tanh: 0.5*tanh(0.5x)+0.5 (stays in exp_and_others set)"""
    dst = ap if out is None else out
    nc.scalar.activation(dst, ap, AF.Tanh, scale=0.5)
    nc.vector.tensor_scalar(dst, dst, 0.5, 0.5, ALU.mult, ALU.add)


def build_nc(T=T_FULL, DEC=DEC_FULL, num_devices=8):
    nc = bacc.Bacc("TRN2", target_bir_lowering=False, debug=False,
                   num_devices=num_devices)

    def inp(name, shape, dt=MM_DT):
        return nc.declare_dram_parameter(name, list(shape), dt, isOutput=False)

    d_wih, d_whh, d_b = [], [], []
    for l in range(NUM_LAYERS):
        din = FEAT if l == 0 else 2 * H
        for d in range(2):
            d_wih.append(inp(f"wih_{l}_{d}", (din, 4 * H)))
            d_whh.append(inp(f"whh_{l}_{d}", (H, 4 * H)))
            d_b.append(inp(f"b_{l}_{d}", (1, 4 * H), F32))
    d_xt0 = inp("xt0", (FEAT, T))
    d_maskb = inp("maskb", (128, T), F32)
    d_maskneg = inp("maskneg", (1, T), F32)
    d_atts = inp("atts_w", (H, 2 * H))
    d_atth = inp("atth_w", (2 * H, 2 * H))
    d_atthb = inp("atth_b", (1, 2 * H), F32)
    d_attw = inp("attw_w", (2 * H, 1))
    d_sy = inp("sy_w", (H, H))
    d_gy = inp("gy_w", (2 * H, H))
    d_yy = inp("yy_w", (H, NYP))
    d_ys = inp("ys_w", (NYP, 4 * H))
    d_ss = inp("ss_w", (H, 4 * H))
    d_gs = inp("gs_w", (2 * H, 4 * H))
    d_sn = inp("sn_w", (H, H))
    d_nn = inp("nn_w", (H, H))
    d_wn = inp("wn_w", (H, H))
    d_dan = inp("dan_w", (NA, H))
    d_nda = inp("nda_w", (H, NA))
    d_ndab = inp("nda_b", (1, NA), F32)
    d_bias2 = inp("bias2", (2, 128), F32)
    d_bias10 = inp("bias10", (10, 128), F32)
    d_yyb = inp("yyb", (4, 128), F32)
    d_wnb = inp("wnb", (2, 128), F32)
    d_ident = inp("ident", (128, 128), F32)
    d_identb = inp("identb", (128, 128))
    d_iotay = inp("iota_y", (1, NYP), F32)
    d_iotaa = inp("iota_a", (1, NA), F32)
    d_ones = inp("onesrow", (1, T), F32)

    d_yout = nc.declare_dram_parameter("yout", [1, DEC], F32, isOutput=True)
    d_Yout = nc.declare_dram_parameter("Yout", [1, DEC], F32, isOutput=True)

    nch = (T + 127) // 128  # time chunks for transposes

    with TileContext(nc) as tc:
        with tc.tile_pool(name="const", bufs=1) as cp, \
             tc.tile_pool(name="xbuf", bufs=2) as xp, \
             tc.tile_pool(name="state", bufs=1) as st, \
             tc.tile_pool(name="work", bufs=4) as wk:

            msk = cp.tile([128, T], F32)
            nc.sync.dma_start(out=msk[:], in_=d_maskb[:])
            mskneg = cp.tile([1, T], F32)
            nc.sync.dma_start(out=mskneg[:], in_=d_maskneg[:])
            ones = cp.tile([1, T], F32)
            nc.sync.dma_start(out=ones[:], in_=d_ones[:])
            ident = cp.tile([128, 128], F32)
            nc.sync.dma_start(out=ident[:], in_=d_ident[:])
            identb = cp.tile([128, 128], MM_DT)
            nc.sync.dma_start(out=identb[:], in_=d_identb[:])
            iotay = cp.tile([1, NYP], F32)
            nc.sync.dma_start(out=iotay[:], in_=d_iotay[:])
            iotaa = cp.tile([1, NA], F32)
            nc.sync.dma_start(out=iotaa[:], in_=d_iotaa[:])

            # ================= ENCODER =================
            x0 = xp.tile([128, 1, T], MM_DT, tag="x0")
            nc.sync.dma_start(out=x0[0:FEAT, 0, :], in_=d_xt0[:])
            xcur = x0
            with tc.tile_pool(name="encw", bufs=2) as wp, \
                 tc.tile_pool(name="encbig", bufs=1) as bigp, \
                 tc.tile_pool(name="eps", bufs=2, space="PSUM") as eps, \
                 tc.tile_pool(name="epre", bufs=2, space="PSUM") as epre:
                for l in range(NUM_LAYERS):
                    din = FEAT if l == 0 else 2 * H
                    nK = (din + 127) // 128
                    kp = [min(128, din - 128 * k) for k in range(nK)]

                    wih = [wp.tile([128, nK, 4 * H], MM_DT, tag=f"wih{d}", name=f"wih{l}{d}") for d in range(2)]
                    whh = [wp.tile([128, 2, 4 * H], MM_DT, tag=f"whh{d}", name=f"whh{l}{d}") for d in range(2)]
                    bb = [wp.tile([1, 4 * H], F32, tag=f"bb{d}", name=f"bb{l}{d}") for d in range(2)]
                    for d in range(2):
                        src = d_wih[2 * l + d]
                        for k in range(nK):
                            nc.sync.dma_start(
                                out=wih[d][0:kp[k], k, :],
                                in_=src[128 * k : 128 * k + kp[k], :])
                        nc.sync.dma_start(
                            out=whh[d][:],
                            in_=d_whh[2 * l + d].rearrange("(k p) m -> p k m", p=128))
                        nc.sync.dma_start(out=bb[d][:], in_=d_b[2 * l + d][:])

                    # xz: (128, 16, T); cols [fwd i,i,f,f,o,o,g,g | bwd same]
                    xz = bigp.tile([128, 16, T], F32, tag="xz")
                    for d in range(2):
                        for gi, g0 in enumerate(GATE_MCOLS):
                            for jj in range(2):
                                col = d * 8 + gi * 2 + jj
                                mc = slice(g0 + jj * 128, g0 + (jj + 1) * 128)
                                acc = epre.tile([128, T], F32, tag="pre")
                                nc.tensor.matmul(acc[:], bb[d][0:1, mc], ones[:],
                                                 start=True, stop=False)
                                for k in range(nK):
                                    nc.tensor.matmul(
                                        acc[:],
                                        wih[d][0:kp[k], k, mc],
                                        xcur[0:kp[k], k, :],
                                        start=False, stop=(k == nK - 1))
                                nc.vector.tensor_copy(xz[:, col, :], acc[:])

                    xnext = xp.tile([128, 4, T], MM_DT, tag="xn")
                    hbf = st.tile([128, 2, 2], MM_DT, tag="hbf")
                    c = st.tile([128, 2, 2], F32, tag="c")
                    nc.vector.memset(hbf[:], 0.0)
                    nc.vector.memset(c[:], 0.0)

                    # two independent per-direction chains -> Tile overlaps
                    # dir-b matmuls with dir-f gate math and vice versa
                    for t in range(T):
                        tt = [t, T - 1 - t]
                        for d in range(2):
                            td = tt[d]
                            z = eps.tile([128, 8], F32, tag=f"z{d}", name=f"z{d}_{l}_{t}")
                            for gi, g0 in enumerate(GATE_MCOLS):
                                for jj in range(2):
                                    for k in range(2):
                                        nc.tensor.matmul(
                                            z[:, gi * 2 + jj : gi * 2 + jj + 1],
                                            whh[d][:, k, g0 + jj * 128 : g0 + (jj + 1) * 128],
                                            hbf[:, d, k : k + 1],
                                            start=(k == 0), stop=(k == 1))
                            zz = wk.tile([128, 8], F32, tag=f"zz{d}", name=f"zz{d}_{l}_{t}")
                            nc.vector.tensor_tensor(zz[:], z[:],
                                                    xz[:, d * 8 : d * 8 + 8, td], ALU.add)
                            _sig(nc, zz[:, 0:6])
                            nc.scalar.activation(zz[:, 6:8], zz[:, 6:8], AF.Tanh)
                            tmp = wk.tile([128, 2], F32, tag=f"tmp{d}", name=f"tmp{d}_{l}_{t}")
                            nc.vector.tensor_tensor(tmp[:], zz[:, 0:2], zz[:, 6:8], ALU.mult)
                            nc.vector.tensor_tensor(c[:, d, :], zz[:, 2:4], c[:, d, :], ALU.mult)
                            nc.vector.tensor_tensor(c[:, d, :], c[:, d, :], tmp[:], ALU.add)
                            tct = wk.tile([128, 2], F32, tag=f"tct{d}", name=f"tct{d}_{l}_{t}")
                            nc.scalar.activation(tct[:], c[:, d, :], AF.Tanh)
                            nc.vector.tensor_tensor(hbf[:, d, :], zz[:, 4:6], tct[:], ALU.mult)
                            if d == 0:
                                nc.vector.tensor_scalar(xnext[:, 0:2, td], hbf[:, 0, :],
                                                        msk[:, td : td + 1], None, ALU.mult)
                            else:
                                nc.vector.tensor_scalar(hbf[:, 1, :], hbf[:, 1, :],
                                                        msk[:, td : td + 1], None, ALU.mult)
                                nc.vector.tensor_scalar(c[:, 1, :], c[:, 1, :],
                                                        msk[:, td : td + 1], None, ALU.mult)
                                nc.vector.tensor_copy(xnext[:, 2:4, td], hbf[:, 1, :])
                    xcur = xnext

            hpack = xcur  # (128, 4, T) f32, k-partition layout, masked

            # ================= DECODER =================
            with tc.tile_pool(name="decw", bufs=1) as dw, \
                 tc.tile_pool(name="decbig", bufs=1) as db, \
                 tc.tile_pool(name="dlin", bufs=1, space="PSUM") as dlin, \
                 tc.tile_pool(name="dscr", bufs=4, space="PSUM") as dscr:

                hp_bf = hpack  # already bf16
                watth = dw.tile([128, 4, 2 * H], MM_DT, tag="watth")
                nc.sync.dma_start(out=watth[:],
                                  in_=d_atth.rearrange("(k p) m -> p k m", p=128))
                batth = dw.tile([1, 2 * H], F32, tag="batth")
                nc.sync.dma_start(out=batth[:], in_=d_atthb[:])

                ah = db.tile([128, 4, T], F32, tag="ah")
                for m in range(4):
                    mc = slice(m * 128, (m + 1) * 128)
                    acc = dscr.tile([128, T], F32, tag="scr")
                    nc.tensor.matmul(acc[:], batth[0:1, mc], ones[:],
                                     start=True, stop=False)
                    for k in range(4):
                        nc.tensor.matmul(acc[:], watth[:, k, mc], hp_bf[:, k, :],
                                         start=False, stop=(k == 3))
                    nc.vector.tensor_copy(ah[:, m, :], acc[:])

                hp_t = db.tile([128, nch, 2 * H], MM_DT, tag="hpt")
                for cch in range(nch):
                    cw = min(128, T - cch * 128)
                    for j in range(4):
                        tp = dscr.tile([128, 128], MM_DT, tag="scr")
                        nc.tensor.matmul(tp[0:cw, 0:128],
                                         hp_bf[:, j, cch * 128 : cch * 128 + cw],
                                         identb[:], is_transpose=True,
                                         start=True, stop=True)
                        nc.vector.tensor_copy(
                            hp_t[0:cw, cch, j * 128 : (j + 1) * 128], tp[0:cw, 0:128])

                def wtile(name, dram, kdim, mdim):
                    nK2 = (kdim + 127) // 128
                    tl = dw.tile([128, nK2, mdim], MM_DT, tag=name)
                    if kdim % 128 == 0:
                        nc.sync.dma_start(out=tl[:],
                                          in_=dram.rearrange("(k p) m -> p k m", p=128))
                    else:
                        nc.sync.dma_start(out=tl[0:kdim, 0, :], in_=dram[:])
                    return tl

                w_atts = wtile("w_atts", d_atts, H, 2 * H)
                w_sy = wtile("w_sy", d_sy, H, H)
                w_ss = wtile("w_ss", d_ss, H, 4 * H)
                w_sn = wtile("w_sn", d_sn, H, H)
                w_gy = wtile("w_gy", d_gy, 2 * H, H)
                w_gs = wtile("w_gs", d_gs, 2 * H, 4 * H)
                w_yy = wtile("w_yy", d_yy, H, NYP)
                w_ys = wtile("w_ys", d_ys, NYP, 4 * H)
                w_nn = wtile("w_nn", d_nn, H, H)
                w_wn = wtile("w_wn", d_wn, H, H)
                w_dan = wtile("w_dan", d_dan, NA, H)
                w_nda = wtile("w_nda", d_nda, H, NA)
                w_attw = dw.tile([128, 4, 1], MM_DT, tag="w_attw")
                nc.sync.dma_start(out=w_attw[:],
                                  in_=d_attw.rearrange("(k p) m -> p k m", p=128))

                bias2 = dw.tile([2, 128], F32, tag="bias2")
                nc.sync.dma_start(out=bias2[:], in_=d_bias2[:])
                bias10 = dw.tile([10, 128], F32, tag="bias10")
                nc.sync.dma_start(out=bias10[:], in_=d_bias10[:])
                yyb = dw.tile([4, 128], F32, tag="yyb")
                nc.sync.dma_start(out=yyb[:], in_=d_yyb[:])
                wnb = dw.tile([2, 128], F32, tag="wnb")
                nc.sync.dma_start(out=wnb[:], in_=d_wnb[:])
                ndab = dw.tile([1, NA], F32, tag="ndab")
                nc.sync.dma_start(out=ndab[:], in_=d_ndab[:])
                onec = dw.tile([1, 1], F32, tag="onec")
                nc.vector.memset(onec[:], 1.0)
                lnS0 = dw.tile([1, 1], F32, tag="lnS0")
                nc.vector.memset(lnS0[:], 3.7612)

                sbf = st.tile([128, 2], MM_DT, tag="sbf")
                nbf = st.tile([128, 2], MM_DT, tag="nbf")
                c2 = st.tile([128, 2], F32, tag="c2")
                nc.vector.memset(sbf[:], 0.0)
                nc.vector.memset(nbf[:], 0.0)
                nc.vector.memset(c2[:], 0.0)

                youts = st.tile([1, DEC], F32, tag="youts")
                Youts = st.tile([1, DEC], F32, tag="Youts")

                for it in range(DEC):
                    # q_ps (4) | yp_ps (2) | zn_ps: [z(8) npre(2)]
                    q_ps = dlin.tile([128, 4], F32, tag="qps")
                    for m in range(4):
                        for k in range(2):
                            nc.tensor.matmul(q_ps[:, m : m + 1],
                                             w_atts[:, k, m * 128 : (m + 1) * 128],
                                             sbf[:, k : k + 1],
                                             start=(k == 0), stop=(k == 1))
                    yp_ps = dlin.tile([128, 2], F32, tag="ypps")
                    nc.tensor.matmul(yp_ps[:], bias2[:], ident[0:2, 0:2],
                                     start=True, stop=False)
                    zn_ps = dlin.tile([128, 10], F32, tag="znps")
                    nc.tensor.matmul(zn_ps[:], bias10[:], ident[0:10, 0:10],
                                     start=True, stop=False)
                    for k in range(2):
                        rh = sbf[:, k : k + 1]
                        for m in range(2):
                            nc.tensor.matmul(yp_ps[:, m : m + 1],
                                             w_sy[:, k, m * 128 : (m + 1) * 128], rh,
                                             start=False, stop=False)
                        for m in range(8):
                            zc = GATE_MCOLS[m // 2] + (m % 2) * 128
                            nc.tensor.matmul(zn_ps[:, m : m + 1],
                                             w_ss[:, k, zc : zc + 128], rh,
                                             start=False, stop=False)
                        for m in range(2):
                            nc.tensor.matmul(zn_ps[:, 8 + m : 9 + m],
                                             w_sn[:, k, m * 128 : (m + 1) * 128], rh,
                                             start=False, stop=False)
                    qsb = wk.tile([128, 4], F32, tag="qsb")
                    nc.vector.tensor_copy(qsb[:], q_ps[:])

                    th = wk.tile([128, 4, T], MM_DT, tag="th")
                    for j in range(4):
                        nc.scalar.activation(th[:, j, :], ah[:, j, :], AF.Tanh,
                                             bias=qsb[:, j : j + 1])
                    e_ps = dscr.tile([1, T], F32, tag="scr")
                    nc.tensor.matmul(e_ps[:], onec[:], mskneg[:], start=True, stop=False)
                    for j in range(4):
                        nc.tensor.matmul(e_ps[:], w_attw[:, j, :], th[:, j, :],
                                         start=False, stop=(j == 3))
                    en = wk.tile([1, T], F32, tag="en")
                    nc.scalar.activation(en[:], e_ps[:], AF.Exp)
                    S = wk.tile([1, 1], F32, tag="S")
                    nc.vector.tensor_reduce(S[:], en[:], AX.X, ALU.add)
                    rS = wk.tile([1, 1], F32, tag="rS")
                    nc.vector.reciprocal(rS[:], S[:])
                    enT = dscr.tile([128, nch], F32, tag="scr")
                    for cch in range(nch):
                        cw = min(128, T - cch * 128)
                        nc.tensor.matmul(enT[0:cw, cch : cch + 1],
                                         en[0:1, cch * 128 : cch * 128 + cw],
                                         ident[0:1, 0:1], is_transpose=True,
                                         start=True, stop=True)
                    enb = wk.tile([128, nch], MM_DT, tag="enb")
                    for cch in range(nch):
                        cw = min(128, T - cch * 128)
                        nc.vector.tensor_copy(enb[0:cw, cch : cch + 1],
                                              enT[0:cw, cch : cch + 1])
                    g_ps = dscr.tile([1, 2 * H], F32, tag="scr")
                    for cch in range(nch):
                        cw = min(128, T - cch * 128)
                        nc.tensor.matmul(g_ps[:], enb[0:cw, cch : cch + 1],
                                         hp_t[0:cw, cch, :],
                                         start=(cch == 0), stop=(cch == nch - 1))
                    gsc = wk.tile([1, 2 * H], F32, tag="gsc")
                    nc.vector.tensor_scalar(gsc[:], g_ps[:], rS[:], None, ALU.mult)
                    gT = dscr.tile([128, 4], F32, tag="scr")
                    for j in range(4):
                        nc.tensor.matmul(gT[:, j : j + 1],
                                         gsc[0:1, j * 128 : (j + 1) * 128],
                                         ident[0:1, 0:1], is_transpose=True,
                                         start=True, stop=True)
                    gbf = wk.tile([128, 4], MM_DT, tag="gbf")
                    nc.vector.tensor_copy(gbf[:], gT[:])

                    for k in range(4):
                        rh = gbf[:, k : k + 1]
                        for m in range(2):
                            nc.tensor.matmul(yp_ps[:, m : m + 1],
                                             w_gy[:, k, m * 128 : (m + 1) * 128], rh,
                                             start=False, stop=(k == 3 and m == 1))
                        for m in range(8):
                            zc = GATE_MCOLS[m // 2] + (m % 2) * 128
                            nc.tensor.matmul(zn_ps[:, m : m + 1],
                                             w_gs[:, k, zc : zc + 128], rh,
                                             start=False, stop=False)

                    yhat = wk.tile([128, 2], MM_DT, tag="yhat")
                    nc.scalar.activation(yhat[:], yp_ps[:], AF.Tanh)
                    y_ps = dscr.tile([128, 4], F32, tag="scr")
                    nc.tensor.matmul(y_ps[:], yyb[:], ident[0:4, 0:4],
                                     start=True, stop=False)
                    for k in range(2):
                        for m in range(4):
                            nc.tensor.matmul(y_ps[:, m : m + 1],
                                             w_yy[:, k, m * 128 : (m + 1) * 128],
                                             yhat[:, k : k + 1],
                                             start=False, stop=(k == 1 and m == 3))
                    ylb = wk.tile([128, 4], MM_DT, tag="ylb")
                    nc.vector.tensor_copy(ylb[:], y_ps[:])
                    ysb = wk.tile([128, 4], F32, tag="ysb")
                    nc.vector.tensor_copy(ysb[:], y_ps[:])
                    yfree = dscr.tile([1, NYP], F32, tag="scr")
                    for j in range(4):
                        nc.tensor.matmul(yfree[0:1, j * 128 : (j + 1) * 128],
                                         ysb[:, j : j + 1], ident[:, :],
                                         is_transpose=True, start=True, stop=True)
                    ym = wk.tile([1, 1], F32, tag="ym")
                    nc.vector.tensor_reduce(ym[:], yfree[:], AX.X, ALU.max)
                    yam = wk.tile([1, NYP], F32, tag="yam")
                    nc.vector.scalar_tensor_tensor(yam[:], yfree[:], ym[:], iotay[:],
                                                   ALU.is_ge, ALU.mult)
                    yi = wk.tile([1, 1], F32, tag="yi")
                    nc.vector.tensor_reduce(yi[:], yam[:], AX.X, ALU.max)
                    nc.vector.tensor_scalar(youts[:, it : it + 1], yi[:],
                                            -1.0, None, ALU.add)

                    da_ps = dscr.tile([1, NA], F32, tag="scr")
                    nc.tensor.matmul(da_ps[:], onec[:], ndab[:], start=True, stop=False)
                    for k in range(2):
                        nc.tensor.matmul(da_ps[:], nbf[:, k : k + 1],
                                         w_nda[:, k, :], start=False, stop=(k == 1))
                    dm = wk.tile([1, 1], F32, tag="dm")
                    nc.vector.tensor_reduce(dm[:], da_ps[:], AX.X, ALU.max)
                    dam = wk.tile([1, NA], F32, tag="dam")
                    nc.vector.scalar_tensor_tensor(dam[:], da_ps[:], dm[:], iotaa[:],
                                                   ALU.is_ge, ALU.mult)
                    di = wk.tile([1, 1], F32, tag="di")
                    nc.vector.tensor_reduce(di[:], dam[:], AX.X, ALU.max)
                    nc.vector.tensor_scalar(Youts[:, it : it + 1], di[:],
                                            -1.0, None, ALU.add)

                    eda = wk.tile([1, NA], F32, tag="eda")
                    nc.scalar.activation(eda[:], da_ps[:], AF.Exp)
                    Sd = wk.tile([1, 1], F32, tag="Sd")
                    nc.vector.tensor_reduce(Sd[:], eda[:], AX.X, ALU.add)
                    lse = wk.tile([1, 1], F32, tag="lse")
                    nc.vector.tensor_copy(lse[:], lnS0[:])
                    for _ in range(3):
                        u = wk.tile([1, 1], F32, tag="u")
                        nc.scalar.activation(u[:], lse[:], AF.Exp, scale=-1.0)
                        nc.vector.scalar_tensor_tensor(lse[:], u[:], Sd[:], lse[:],
                                                       ALU.mult, ALU.add)
                        nc.vector.tensor_scalar(lse[:], lse[:], -1.0, None, ALU.add)
                    neglse = wk.tile([1, 1], F32, tag="neglse")
                    nc.vector.tensor_scalar(neglse[:], lse[:], -1.0, None, ALU.mult)
                    logp = wk.tile([1, NA], MM_DT, tag="logp")
                    nc.vector.tensor_scalar(logp[:], da_ps[:], neglse[:], None, ALU.add)
                    lpT = dscr.tile([NA, 1], MM_DT, tag="scr")
                    nc.tensor.matmul(lpT[:], logp[0:1, :], identb[0:1, 0:1],
                                     is_transpose=True, start=True, stop=True)
                    lpb = wk.tile([NA, 1], MM_DT, tag="lpb")
                    nc.vector.tensor_copy(lpb[:], lpT[:])

                    for k in range(2):
                        for m in range(2):
                            nc.tensor.matmul(zn_ps[:, 8 + m : 9 + m],
                                             w_nn[:, k, m * 128 : (m + 1) * 128],
                                             nbf[:, k : k + 1], start=False, stop=False)
                    for m in range(2):
                        nc.tensor.matmul(zn_ps[:, 8 + m : 9 + m],
                                         w_dan[0:NA, 0, m * 128 : (m + 1) * 128],
                                         lpb[:], start=False, stop=False)
                    for k in range(4):
                        for m in range(8):
                            zc = GATE_MCOLS[m // 2] + (m % 2) * 128
                            nc.tensor.matmul(zn_ps[:, m : m + 1],
                                             w_ys[:, k, zc : zc + 128],
                                             ylb[:, k : k + 1],
                                             start=False, stop=(k == 3 and m == 7))

                    nhat = wk.tile([128, 2], MM_DT, tag="nhat")
                    nc.scalar.activation(nhat[:], zn_ps[:, 8:10], AF.Tanh)
                    n_ps = dscr.tile([128, 2], F32, tag="scr")
                    nc.tensor.matmul(n_ps[:], wnb[:], ident[0:2, 0:2],
                                     start=True, stop=False)
                    for k in range(2):
                        for m in range(2):
                            nc.tensor.matmul(n_ps[:, m : m + 1],
                                             w_wn[:, k, m * 128 : (m + 1) * 128],
                                             nhat[:, k : k + 1],
                                             start=False, stop=(k == 1 and m == 1))
                    nc.vector.tensor_copy(nbf[:], n_ps[:])

                    zz2 = wk.tile([128, 8], F32, tag="zz2")
                    _sig(nc, zn_ps[:, 0:6], out=zz2[:, 0:6])
                    nc.scalar.activation(zz2[:, 6:8], zn_ps[:, 6:8], AF.Tanh)
                    tmp2 = wk.tile([128, 2], F32, tag="tmp2")
                    nc.vector.tensor_tensor(tmp2[:], zz2[:, 0:2], zz2[:, 6:8], ALU.mult)
                    nc.vector.tensor_tensor(c2[:], zz2[:, 2:4], c2[:], ALU.mult)
                    nc.vector.tensor_tensor(c2[:], c2[:], tmp2[:], ALU.add)
                    tc2 = wk.tile([128, 2], F32, tag="tc2")
                    nc.scalar.activation(tc2[:], c2[:], AF.Tanh)
                    nc.vector.tensor_tensor(sbf[:], zz2[:, 4:6], tc2[:], ALU.mult)

                nc.sync.dma_start(out=d_yout[:], in_=youts[:])
                nc.sync.dma_start(out=d_Yout[:], in_=Youts[:])

    nc.compile()
    return nc


_NC = None


def kernel(data, length, lstm_params, params):
    global _NC
    shared = pack_weights(lstm_params, params)
    per_core = pack_percore(shared, data, length, T_FULL)
    if _NC is None:
        _NC = build_nc()
    res = run_bass_kernel_spmd(_NC, per_core, list(range(B)))
    yout = np.stack([_f32(res.results[b]["yout"])[0] for b in range(B)])
    Yout = np.stack([_f32(res.results[b]["Yout"])[0] for b in range(B)])
    return (yout, Yout)


# revision 14
# speedup vs baseline: 1.5269x; 1.2818x over previous
"""Trainium2 Bass kernel for a 4-layer bidirectional-LSTM encoder +
200-step attention decoder (argmax token / dialog-act outputs).

Sharding: data-parallel over batch. B=8 -> 1 batch element per NeuronCore.
All weights replicated; zero collectives; outputs gathered on host.

Per-core layout strategy:
  - hidden dim H=256 lives on SBUF partitions as (128, 2) column pairs
  - all matvecs use weights as lhsT (stationary) so outputs land on
    partitions; biases are injected with K=n "bias matmuls" against an
    identity stationary built from host-packed (n,128) bias matrices
  - LSTM fwd+bwd directions are batched per step; the bwd direction runs
    on reversed time indices (compile-time constants; fully unrolled)
    with carry-masking to reproduce pack/pad semantics
  - only the exp_and_others ACT table set is used (exp + tanh); sigmoid
    is computed as 0.5*tanh(0.5x)+0.5 and ln() via Newton on exp
"""

import sys

sys.path.insert(0, "/opt/trn_rl_repo")

import numpy as np
import ml_dtypes

import concourse.bass as bass
import concourse.mybir as mybir
from concourse import bacc
from concourse.tile import TileContext
from concourse.bass_utils import run_bass_kernel_spmd

F32 = mybir.dt.float32
BF16 = mybir.dt.bfloat16
AF = mybir.ActivationFunctionType
ALU = mybir.AluOpType
AX = mybir.AxisListType

H = 256
NUM_LAYERS = 4
NY = 500
NYP = 512  # padded
NA = 43
B, FEAT = 8, 120
T_FULL, DEC_FULL = 400, 200
NEG_B = 30.0

MM_DT = BF16
MM_NP = ml_dtypes.bfloat16

GATE_MCOLS = [0, 256, 768, 512]  # i, f, o, g weight-column starts


def _bf(x):
    return np.ascontiguousarray(np.asarray(x, np.float32).astype(MM_NP))


def _f32(x):
    return np.ascontiguousarray(np.asarray(x, np.float32))


# ----------------------------------------------------------------------------
# host-side packing
# ----------------------------------------------------------------------------

def pack_weights(lstm_params, params):
    shared = {}
    for l in range(NUM_LAYERS):
        for d in range(2):
            p = lstm_params[2 * l + d]
            shared[f"wih_{l}_{d}"] = _bf(p["Wih"])
            shared[f"whh_{l}_{d}"] = _bf(p["Whh"])
            shared[f"b_{l}_{d}"] = _f32(np.asarray(p["b"], np.float32)[None, :])
    pr = {k: np.asarray(v, np.float32) for k, v in params.items()}

    shared["atts_w"] = _bf(pr["atts_w"])
    shared["atth_w"] = _bf(pr["atth_w"])
    shared["atth_b"] = _f32(pr["atth_b"][None])
    shared["attw_w"] = _bf(pr["attw_w"])
    shared["sy_w"] = _bf(pr["sy_w"])
    shared["gy_w"] = _bf(pr["gy_w"])
    yy = np.zeros((H, NYP), np.float32)
    yy[:, :NY] = pr["yy_w"]
    shared["yy_w"] = _bf(yy)
    ys = np.zeros((NYP, 4 * H), np.float32)
    ys[:NY] = pr["ys_w"]
    shared["ys_w"] = _bf(ys)
    shared["ss_w"] = _bf(pr["ss_w"])
    shared["gs_w"] = _bf(pr["gs_w"])
    shared["sn_w"] = _bf(pr["sn_w"])
    shared["nn_w"] = _bf(pr["nn_w"])
    shared["wn_w"] = _bf(pr["wn_w"])
    shared["dan_w"] = _bf(pr["dan_w"])
    shared["nda_w"] = _bf(pr["nda_w"])
    shared["nda_b"] = _f32(pr["nda_b"][None])

    shared["bias2"] = _f32(pr["gy_b"].reshape(2, 128))
    b10 = np.zeros((10, 128), np.float32)
    zb = (pr["gs_b"] + pr["ys_b"]).reshape(8, 128)
    b10[0:8] = zb[[0, 1, 2, 3, 6, 7, 4, 5]]
    b10[8:10] = (pr["nn_b"] + pr["dan_b"]).reshape(2, 128)
    shared["bias10"] = _f32(b10)
    yyb = np.full((4, 128), -1.0e9, np.float32)
    yyb.reshape(-1)[:NY] = pr["yy_b"]
    shared["yyb"] = _f32(yyb)
    shared["wnb"] = _f32(pr["wn_b"].reshape(2, 128))

    ident = np.eye(128, dtype=np.float32)
    shared["ident"] = ident
    shared["identb"] = _bf(ident)
    shared["iota_y"] = _f32((1.0 + np.arange(NYP, dtype=np.float32))[None])
    shared["iota_a"] = _f32((1.0 + np.arange(NA, dtype=np.float32))[None])
    return shared


def pack_percore(shared, data, length, T):
    data = np.asarray(data, np.float32)
    length = np.asarray(length, np.int32)
    shared = dict(shared)
    shared["onesrow"] = _f32(np.ones((1, T), np.float32))
    per_core = []
    for b in range(data.shape[0]):
        m = dict(shared)
        m["xt0"] = _bf(data[b].T)
        mask = (np.arange(T) < int(length[b])).astype(np.float32)
        m["maskb"] = _f32(np.broadcast_to(mask[None, :], (128, T)))
        m["maskneg"] = _f32(((mask - 1.0) * NEG_B)[None, :])
        per_core.append(m)
    return per_core


# ----------------------------------------------------------------------------
# bass program
# ----------------------------------------------------------------------------

def _sig(nc, ap, out=None):
    """sigmoid via tanh: 0.5*tanh(0.5x)+0.5 (stays in exp_and_others set)"""
    dst = ap if out is None else out
    nc.scalar.activation(dst, ap, AF.Tanh, scale=0.5)
    nc.vector.tensor_scalar(dst, dst, 0.5, 0.5, ALU.mult, ALU.add)


def build_nc(T=T_FULL, DEC=DEC_FULL, num_devices=8):
    nc = bacc.Bacc("TRN2", target_bir_lowering=False, debug=False,
                   num_devices=num_devices)

    def inp(name, shape, dt=MM_DT):
        return nc.declare_dram_parameter(name, list(shape), dt, isOutput=False)

    d_wih, d_whh, d_b = [], [], []
    for l in range(NUM_LAYERS):
        din = FEAT if l == 0 else 2 * H
        for d in range(2):
            d_wih.append(inp(f"wih_{l}_{d}", (din, 4 * H)))
            d_whh.append(inp(f"whh_{l}_{d}", (H, 4 * H)))
            d_b.append(inp(f"b_{l}_{d}", (1, 4 * H), F32))
    d_xt0 = inp("xt0", (FEAT, T))
    d_maskb = inp("maskb", (128, T), F32)
    d_maskneg = inp("maskneg", (1, T), F32)
    d_atts = inp("atts_w", (H, 2 * H))
    d_atth = inp("atth_w", (2 * H, 2 * H))
    d_atthb = inp("atth_b", (1, 2 * H), F32)
    d_attw = inp("attw_w", (2 * H, 1))
    d_sy = inp("sy_w", (H, H))
    d_gy = inp("gy_w", (2 * H, H))
    d_yy = inp("yy_w", (H, NYP))
    d_ys = inp("ys_w", (NYP, 4 * H))
    d_ss = inp("ss_w", (H, 4 * H))
    d_gs = inp("gs_w", (2 * H, 4 * H))
    d_sn = inp("sn_w", (H, H))
    d_nn = inp("nn_w", (H, H))
    d_wn = inp("wn_w", (H, H))
    d_dan = inp("dan_w", (NA, H))
    d_nda = inp("nda_w", (H, NA))
    d_ndab = inp("nda_b", (1, NA), F32)
    d_bias2 = inp("bias2", (2, 128), F32)
    d_bias10 = inp("bias10", (10, 128), F32)
    d_yyb = inp("yyb", (4, 128), F32)
    d_wnb = inp("wnb", (2, 128), F32)
    d_ident = inp("ident", (128, 128), F32)
    d_identb = inp("identb", (128, 128))
    d_iotay = inp("iota_y", (1, NYP), F32)
    d_iotaa = inp("iota_a", (1, NA), F32)
    d_ones = inp("onesrow", (1, T), F32)

    d_yout = nc.declare_dram_parameter("yout", [1, DEC], F32, isOutput=True)
    d_Yout = nc.declare_dram_parameter("Yout", [1, DEC], F32, isOutput=True)

    nch = (T + 127) // 128  # time chunks for transposes

    with TileContext(nc) as tc:
        with tc.tile_pool(name="const", bufs=1) as cp, \
             tc.tile_pool(name="xbuf", bufs=2) as xp, \
             tc.tile_pool(name="state", bufs=1) as st, \
             tc.tile_pool(name="work", bufs=4) as wk:

            msk = cp.tile([128, T], F32)
            nc.sync.dma_start(out=msk[:], in_=d_maskb[:])
            mskneg = cp.tile([1, T], F32)
            nc.sync.dma_start(out=mskneg[:], in_=d_maskneg[:])
            ones = cp.tile([1, T], F32)
            nc.sync.dma_start(out=ones[:], in_=d_ones[:])
            ident = cp.tile([128, 128], F32)
            nc.sync.dma_start(out=ident[:], in_=d_ident[:])
            identb = cp.tile([128, 128], MM_DT)
            nc.sync.dma_start(out=identb[:], in_=d_identb[:])
            iotay = cp.tile([1, NYP], F32)
            nc.sync.dma_start(out=iotay[:], in_=d_iotay[:])
            iotaa = cp.tile([1, NA], F32)
            nc.sync.dma_start(out=iotaa[:], in_=d_iotaa[:])

            # ================= ENCODER =================
            x0 = xp.tile([128, 1, T], MM_DT, tag="x0")
            nc.sync.dma_start(out=x0[0:FEAT, 0, :], in_=d_xt0[:])
            xcur = x0
            with tc.tile_pool(name="encw", bufs=2) as wp, \
                 tc.tile_pool(name="encbig", bufs=1) as bigp, \
                 tc.tile_pool(name="eps", bufs=2, space="PSUM") as eps, \
                 tc.tile_pool(name="epre", bufs=2, space="PSUM") as epre:
                for l in range(NUM_LAYERS):
                    din = FEAT if l == 0 else 2 * H
                    nK = (din + 127) // 128
                    kp = [min(128, din - 128 * k) for k in range(nK)]

                    wih = [wp.tile([128, nK, 4 * H], MM_DT, tag=f"wih{d}", name=f"wih{l}{d}") for d in range(2)]
                    whh = [wp.tile([128, 2, 4 * H], MM_DT, tag=f"whh{d}", name=f"whh{l}{d}") for d in range(2)]
                    bb = [wp.tile([1, 4 * H], F32, tag=f"bb{d}", name=f"bb{l}{d}") for d in range(2)]
                    for d in range(2):
                        src = d_wih[2 * l + d]
                        for k in range(nK):
                            nc.sync.dma_start(
                                out=wih[d][0:kp[k], k, :],
                                in_=src[128 * k : 128 * k + kp[k], :])
                        nc.sync.dma_start(
                            out=whh[d][:],
                            in_=d_whh[2 * l + d].rearrange("(k p) m -> p k m", p=128))
                        nc.sync.dma_start(out=bb[d][:], in_=d_b[2 * l + d][:])

                    # xz: (128, 16, T); cols [fwd i,i,f,f,o,o,g,g | bwd same]
                    xz = bigp.tile([128, 16, T], F32, tag="xz")
                    for d in range(2):
                        for gi, g0 in enumerate(GATE_MCOLS):
                            for jj in range(2):
                                col = d * 8 + gi * 2 + jj
                                mc = slice(g0 + jj * 128, g0 + (jj + 1) * 128)
                                acc = epre.tile([128, T], F32, tag="pre")
                                nc.tensor.matmul(acc[:], bb[d][0:1, mc], ones[:],
                                                 start=True, stop=False)
                                for k in range(nK):
                                    nc.tensor.matmul(
                                        acc[:],
                                        wih[d][0:kp[k], k, mc],
                                        xcur[0:kp[k], k, :],
                                        start=False, stop=(k == nK - 1))
                                nc.vector.tensor_copy(xz[:, col, :], acc[:])

                    xnext = xp.tile([128, 4, T], MM_DT, tag="xn")
                    hbf = st.tile([128, 2, 2], MM_DT, tag="hbf")
                    c = st.tile([128, 2, 2], F32, tag="c")
                    nc.vector.memset(hbf[:], 0.0)
                    nc.vector.memset(c[:], 0.0)

                    # two independent per-direction chains -> Tile overlaps
                    # dir-b matmuls with dir-f gate math and vice versa
                    for t in range(T):
                        tt = [t, T - 1 - t]
                        for d in range(2):
                            td = tt[d]
                            z = eps.tile([128, 8], F32, tag=f"z{d}", name=f"z{d}_{l}_{t}")
                            for gi, g0 in enumerate(GATE_MCOLS):
                                for jj in range(2):
                                    for k in range(2):
                                        nc.tensor.matmul(
                                            z[:, gi * 2 + jj : gi * 2 + jj + 1],
                                            whh[d][:, k, g0 + jj * 128 : g0 + (jj + 1) * 128],
                                            hbf[:, d, k : k + 1],
                                            start=(k == 0), stop=(k == 1))
                            zz = wk.tile([128, 8], F32, tag=f"zz{d}", name=f"zz{d}_{l}_{t}")
                            nc.vector.tensor_tensor(zz[:], z[:],
                                                    xz[:, d * 8 : d * 8 + 8, td], ALU.add)
                            _sig(nc, zz[:, 0:6])
                            nc.scalar.activation(zz[:, 6:8], zz[:, 6:8], AF.Tanh)
                            tmp = wk.tile([128, 2], F32, tag=f"tmp{d}", name=f"tmp{d}_{l}_{t}")
                            nc.vector.tensor_tensor(tmp[:], zz[:, 0:2], zz[:, 6:8], ALU.mult)
                            nc.vector.tensor_tensor(c[:, d, :], zz[:, 2:4], c[:, d, :], ALU.mult)
                            nc.vector.tensor_tensor(c[:, d, :], c[:, d, :], tmp[:], ALU.add)
                            tct = wk.tile([128, 2], F32, tag=f"tct{d}", name=f"tct{d}_{l}_{t}")
                            nc.scalar.activation(tct[:], c[:, d, :], AF.Tanh)
                            nc.vector.tensor_tensor(hbf[:, d, :], zz[:, 4:6], tct[:], ALU.mult)
                            if d == 0:
                                nc.vector.tensor_scalar(xnext[:, 0:2, td], hbf[:, 0, :],
                                                        msk[:, td : td + 1], None, ALU.mult)
                            else:
                                nc.vector.tensor_scalar(hbf[:, 1, :], hbf[:, 1, :],
                                                        msk[:, td : td + 1], None, ALU.mult)
                                nc.vector.tensor_scalar(c[:, 1, :], c[:, 1, :],
                                                        msk[:, td : td + 1], None, ALU.mult)
                                nc.vector.tensor_copy(xnext[:, 2:4, td], hbf[:, 1, :])
                    xcur = xnext

            hpack = xcur  # (128, 4, T) f32, k-partition layout, masked

            # ================= DECODER =================
            with tc.tile_pool(name="decw", bufs=1) as dw, \
                 tc.tile_pool(name="decbig", bufs=1) as db, \
                 tc.tile_pool(name="dlin", bufs=1, space="PSUM") as dlin, \
                 tc.tile_pool(name="dscr", bufs=4, space="PSUM") as dscr:

                hp_bf = hpack  # already bf16
                watth = dw.tile([128, 4, 2 * H], MM_DT, tag="watth")
                nc.sync.dma_start(out=watth[:],
                                  in_=d_atth.rearrange("(k p) m -> p k m", p=128))
                batth = dw.tile([1, 2 * H], F32, tag="batth")
                nc.sync.dma_start(out=batth[:], in_=d_atthb[:])

                ah = db.tile([128, 4, T], F32, tag="ah")
                for m in range(4):
                    mc = slice(m * 128, (m + 1) * 128)
                    acc = dscr.tile([128, T], F32, tag="scr")
                    nc.tensor.matmul(acc[:], batth[0:1, mc], ones[:],
                                     start=True, stop=False)
                    for k in range(4):
                        nc.tensor.matmul(acc[:], watth[:, k, mc], hp_bf[:, k, :],
                                         start=False, stop=(k == 3))
                    nc.vector.tensor_copy(ah[:, m, :], acc[:])

                hp_t = db.tile([128, nch, 2 * H], MM_DT, tag="hpt")
                for cch in range(nch):
                    cw = min(128, T - cch * 128)
                    for j in range(4):
                        tp = dscr.tile([128, 128], MM_DT, tag="scr")
                        nc.tensor.matmul(tp[0:cw, 0:128],
                                         hp_bf[:, j, cch * 128 : cch * 128 + cw],
                                         identb[:], is_transpose=True,
                                         start=True, stop=True)
                        nc.vector.tensor_copy(
                            hp_t[0:cw, cch, j * 128 : (j + 1) * 128], tp[0:cw, 0:128])

                def wtile(name, dram, kdim, mdim):
                    nK2 = (kdim + 127) // 128
                    tl = dw.tile([128, nK2, mdim], MM_DT, tag=name)
                    if kdim % 128 == 0:
                        nc.sync.dma_start(out=tl[:],
                                          in_=dram.rearrange("(k p) m -> p k m", p=128))
                    else:
                        nc.sync.dma_start(out=tl[0:kdim, 0, :], in_=dram[:])
                    return tl

                w_atts = wtile("w_atts", d_atts, H, 2 * H)
                w_sy = wtile("w_sy", d_sy, H, H)
                w_ss = wtile("w_ss", d_ss, H, 4 * H)
                w_sn = wtile("w_sn", d_sn, H, H)
                w_gy = wtile("w_gy", d_gy, 2 * H, H)
                w_gs = wtile("w_gs", d_gs, 2 * H, 4 * H)
                w_yy = wtile("w_yy", d_yy, H, NYP)
                w_ys = wtile("w_ys", d_ys, NYP, 4 * H)
                w_nn = wtile("w_nn", d_nn, H, H)
                w_wn = wtile("w_wn", d_wn, H, H)
                w_dan = wtile("w_dan", d_dan, NA, H)
                w_nda = wtile("w_nda", d_nda, H, NA)
                w_attw = dw.tile([128, 4, 1], MM_DT, tag="w_attw")
                nc.sync.dma_start(out=w_attw[:],
                                  in_=d_attw.rearrange("(k p) m -> p k m", p=128))

                bias2 = dw.tile([2, 128], F32, tag="bias2")
                nc.sync.dma_start(out=bias2[:], in_=d_bias2[:])
                bias10 = dw.tile([10, 128], F32, tag="bias10")
                nc.sync.dma_start(out=bias10[:], in_=d_bias10[:])
                yyb = dw.tile([4, 128], F32, tag="yyb")
                nc.sync.dma_start(out=yyb[:], in_=d_yyb[:])
                wnb = dw.tile([2, 128], F32, tag="wnb")
                nc.sync.dma_start(out=wnb[:], in_=d_wnb[:])
                ndab = dw.tile([1, NA], F32, tag="ndab")
                nc.sync.dma_start(out=ndab[:], in_=d_ndab[:])
                onec = dw.tile([1, 1], F32, tag="onec")
                nc.vector.memset(onec[:], 1.0)
                lnS0 = dw.tile([1, 1], F32, tag="lnS0")
                nc.vector.memset(lnS0[:], 3.7612)

                sbf = st.tile([128, 2], MM_DT, tag="sbf")
                nbf = st.tile([128, 2], MM_DT, tag="nbf")
                c2 = st.tile([128, 2], F32, tag="c2")
                nc.vector.memset(sbf[:], 0.0)
                nc.vector.memset(nbf[:], 0.0)
                nc.vector.memset(c2[:], 0.0)

                youts = st.tile([1, DEC], F32, tag="youts")
                Youts = st.tile([1, DEC], F32, tag="Youts")

                for it in range(DEC):
                    # q_ps (4) | yp_ps (2) | zn_ps: [z(8) npre(2)]
                    q_ps = dlin.tile([128, 4], F32, tag="qps")
                    for m in range(4):
                        for k in range(2):
                            nc.tensor.matmul(q_ps[:, m : m + 1],
                                             w_atts[:, k, m * 128 : (m + 1) * 128],
                                             sbf[:, k : k + 1],
                                             start=(k == 0), stop=(k == 1))
                    yp_ps = dlin.tile([128, 2], F32, tag="ypps")
                    nc.tensor.matmul(yp_ps[:], bias2[:], ident[0:2, 0:2],
                                     start=True, stop=False)
                    zn_ps = dlin.tile([128, 10], F32, tag="znps")
                    nc.tensor.matmul(zn_ps[:], bias10[:], ident[0:10, 0:10],
                                     start=True, stop=False)
                    for k in range(2):
                        rh = sbf[:, k : k + 1]
                        for m in range(2):
                            nc.tensor.matmul(yp_ps[:, m : m + 1],
                                             w_sy[:, k, m * 128 : (m + 1) * 128], rh,
                                             start=False, stop=False)
                        for m in range(8):
                            zc = GATE_MCOLS[m // 2] + (m % 2) * 128
                            nc.tensor.matmul(zn_ps[:, m : m + 1],
                                             w_ss[:, k, zc : zc + 128], rh,
                                             start=False, stop=False)
                        for m in range(2):
                            nc.tensor.matmul(zn_ps[:, 8 + m : 9 + m],
                                             w_sn[:, k, m * 128 : (m + 1) * 128], rh,
                                             start=False, stop=False)
                    qsb = wk.tile([128, 4], F32, tag="qsb")
                    for j in range(4):
                        nc.vector.tensor_copy(qsb[:, j : j + 1], q_ps[:, j : j + 1])

                    th = wk.tile([128, 4, T], MM_DT, tag="th")
                    for j in range(4):
                        nc.scalar.activation(th[:, j, :], ah[:, j, :], AF.Tanh,
                                             bias=qsb[:, j : j + 1])
                    e_ps = dscr.tile([1, T], F32, tag="scr")
                    nc.tensor.matmul(e_ps[:], onec[:], mskneg[:], start=True, stop=False)
                    for j in range(4):
                        nc.tensor.matmul(e_ps[:], w_attw[:, j, :], th[:, j, :],
                                         start=False, stop=(j == 3))
                    en = wk.tile([1, T], F32, tag="en")
                    nc.scalar.activation(en[:], e_ps[:], AF.Exp)
                    S = wk.tile([1, 1], F32, tag="S")
                    nc.vector.tensor_reduce(S[:], en[:], AX.X, ALU.add)
                    rS = wk.tile([1, 1], F32, tag="rS")
                    nc.vector.reciprocal(rS[:], S[:])
                    # normalize in the 1-partition layout, then transpose:
                    # alpha = en/S, so downstream g needs no rescale
                    ens = wk.tile([1, T], F32, tag="ens")
                    nc.vector.tensor_scalar(ens[:], en[:], rS[:], None, ALU.mult)
                    enT = dscr.tile([128, nch], F32, tag="scr")
                    for cch in range(nch):
                        cw = min(128, T - cch * 128)
                        nc.tensor.matmul(enT[0:cw, cch : cch + 1],
                                         ens[0:1, cch * 128 : cch * 128 + cw],
                                         ident[0:1, 0:1], is_transpose=True,
                                         start=True, stop=True)
                    enb = wk.tile([128, nch], MM_DT, tag="enb")
                    for cch in range(nch):
                        cw = min(128, T - cch * 128)
                        nc.vector.tensor_copy(enb[0:cw, cch : cch + 1],
                                              enT[0:cw, cch : cch + 1])
                    # g directly partition-major: lhsT = hp_t chunks (t-part, k-free)
                    gT = dscr.tile([128, 4], F32, tag="scr")
                    for m in range(4):
                        for cch in range(nch):
                            cw = min(128, T - cch * 128)
                            nc.tensor.matmul(gT[:, m : m + 1],
                                             hp_t[0:cw, cch, m * 128 : (m + 1) * 128],
                                             enb[0:cw, cch : cch + 1],
                                             start=(cch == 0), stop=(cch == nch - 1))
                    gbf = wk.tile([128, 4], MM_DT, tag="gbf")
                    nc.vector.tensor_copy(gbf[:], gT[:])

                    for k in range(4):
                        rh = gbf[:, k : k + 1]
                        for m in range(2):
                            nc.tensor.matmul(yp_ps[:, m : m + 1],
                                             w_gy[:, k, m * 128 : (m + 1) * 128], rh,
                                             start=False, stop=(k == 3 and m == 1))
                        for m in range(8):
                            zc = GATE_MCOLS[m // 2] + (m % 2) * 128
                            nc.tensor.matmul(zn_ps[:, m : m + 1],
                                             w_gs[:, k, zc : zc + 128], rh,
                                             start=False, stop=False)

                    yhat = wk.tile([128, 2], MM_DT, tag="yhat")
                    nc.scalar.activation(yhat[:], yp_ps[:], AF.Tanh)
                    y_ps = dscr.tile([128, 4], F32, tag="scr")
                    nc.tensor.matmul(y_ps[:], yyb[:], ident[0:4, 0:4],
                                     start=True, stop=False)
                    for k in range(2):
                        for m in range(4):
                            nc.tensor.matmul(y_ps[:, m : m + 1],
                                             w_yy[:, k, m * 128 : (m + 1) * 128],
                                             yhat[:, k : k + 1],
                                             start=False, stop=(k == 1 and m == 3))
                    ylb = wk.tile([128, 4], MM_DT, tag="ylb")
                    nc.vector.tensor_copy(ylb[:], y_ps[:])
                    ysb = wk.tile([128, 4], F32, tag="ysb")
                    nc.vector.tensor_copy(ysb[:], y_ps[:])
                    yfree = dscr.tile([1, NYP], F32, tag="scr")
                    for j in range(4):
                        nc.tensor.matmul(yfree[0:1, j * 128 : (j + 1) * 128],
                                         ysb[:, j : j + 1], ident[:, :],
                                         is_transpose=True, start=True, stop=True)
                    ym = wk.tile([1, 1], F32, tag="ym")
                    nc.vector.tensor_reduce(ym[:], yfree[:], AX.X, ALU.max)
                    yam = wk.tile([1, NYP], F32, tag="yam")
                    nc.vector.scalar_tensor_tensor(yam[:], yfree[:], ym[:], iotay[:],
                                                   ALU.is_ge, ALU.mult)
                    yi = wk.tile([1, 1], F32, tag="yi")
                    nc.vector.tensor_reduce(yi[:], yam[:], AX.X, ALU.max)
                    nc.vector.tensor_scalar(youts[:, it : it + 1], yi[:],
                                            -1.0, None, ALU.add)

                    da_ps = dscr.tile([1, NA], F32, tag="scr")
                    nc.tensor.matmul(da_ps[:], onec[:], ndab[:], start=True, stop=False)
                    for k in range(2):
                        nc.tensor.matmul(da_ps[:], nbf[:, k : k + 1],
                                         w_nda[:, k, :], start=False, stop=(k == 1))
                    dm = wk.tile([1, 1], F32, tag="dm")
                    nc.vector.tensor_reduce(dm[:], da_ps[:], AX.X, ALU.max)
                    dam = wk.tile([1, NA], F32, tag="dam")
                    nc.vector.scalar_tensor_tensor(dam[:], da_ps[:], dm[:], iotaa[:],
                                                   ALU.is_ge, ALU.mult)
                    di = wk.tile([1, 1], F32, tag="di")
                    nc.vector.tensor_reduce(di[:], dam[:], AX.X, ALU.max)
                    nc.vector.tensor_scalar(Youts[:, it : it + 1], di[:],
                                            -1.0, None, ALU.add)

                    eda = wk.tile([1, NA], F32, tag="eda")
                    nc.scalar.activation(eda[:], da_ps[:], AF.Exp)
                    Sd = wk.tile([1, 1], F32, tag="Sd")
                    nc.vector.tensor_reduce(Sd[:], eda[:], AX.X, ALU.add)
                    lse = wk.tile([1, 1], F32, tag="lse")
                    nc.vector.tensor_copy(lse[:], lnS0[:])
                    for _ in range(3):
                        u = wk.tile([1, 1], F32, tag="u")
                        nc.scalar.activation(u[:], lse[:], AF.Exp, scale=-1.0)
                        nc.vector.scalar_tensor_tensor(lse[:], u[:], Sd[:], lse[:],
                                                       ALU.mult, ALU.add)
                        nc.vector.tensor_scalar(lse[:], lse[:], -1.0, None, ALU.add)
                    neglse = wk.tile([1, 1], F32, tag="neglse")
                    nc.vector.tensor_scalar(neglse[:], lse[:], -1.0, None, ALU.mult)
                    logp = wk.tile([1, NA], MM_DT, tag="logp")
                    nc.vector.tensor_scalar(logp[:], da_ps[:], neglse[:], None, ALU.add)
                    lpT = dscr.tile([NA, 1], MM_DT, tag="scr")
                    nc.tensor.matmul(lpT[:], logp[0:1, :], identb[0:1, 0:1],
                                     is_transpose=True, start=True, stop=True)
                    lpb = wk.tile([NA, 1], MM_DT, tag="lpb")
                    nc.vector.tensor_copy(lpb[:], lpT[:])

                    for k in range(2):
                        for m in range(2):
                            nc.tensor.matmul(zn_ps[:, 8 + m : 9 + m],
                                             w_nn[:, k, m * 128 : (m + 1) * 128],
                                             nbf[:, k : k + 1], start=False, stop=False)
                    for m in range(2):
                        nc.tensor.matmul(zn_ps[:, 8 + m : 9 + m],
                                         w_dan[0:NA, 0, m * 128 : (m + 1) * 128],
                                         lpb[:], start=False, stop=False)
                    for k in range(4):
                        for m in range(8):
                            zc = GATE_MCOLS[m // 2] + (m % 2) * 128
                            nc.tensor.matmul(zn_ps[:, m : m + 1],
                                             w_ys[:, k, zc : zc + 128],
                                             ylb[:, k : k + 1],
                                             start=False, stop=(k == 3 and m == 7))

                    nhat = wk.tile([128, 2], MM_DT, tag="nhat")
                    nc.scalar.activation(nhat[:], zn_ps[:, 8:10], AF.Tanh)
                    n_ps = dscr.tile([128, 2], F32, tag="scr")
                    nc.tensor.matmul(n_ps[:], wnb[:], ident[0:2, 0:2],
                                     start=True, stop=False)
                    for k in range(2):
                        for m in range(2):
                            nc.tensor.matmul(n_ps[:, m : m + 1],
                                             w_wn[:, k, m * 128 : (m + 1) * 128],
                                             nhat[:, k : k + 1],
                                             start=False, stop=(k == 1 and m == 1))
                    nc.vector.tensor_copy(nbf[:], n_ps[:])

                    zz2 = wk.tile([128, 8], F32, tag="zz2")
                    _sig(nc, zn_ps[:, 0:6], out=zz2[:, 0:6])
                    nc.scalar.activation(zz2[:, 6:8], zn_ps[:, 6:8], AF.Tanh)
                    tmp2 = wk.tile([128, 2], F32, tag="tmp2")
                    nc.vector.tensor_tensor(tmp2[:], zz2[:, 0:2], zz2[:, 6:8], ALU.mult)
                    nc.vector.tensor_tensor(c2[:], zz2[:, 2:4], c2[:], ALU.mult)
                    nc.vector.tensor_tensor(c2[:], c2[:], tmp2[:], ALU.add)
                    tc2 = wk.tile([128, 2], F32, tag="tc2")
                    nc.scalar.activation(tc2[:], c2[:], AF.Tanh)
                    nc.vector.tensor_tensor(sbf[:], zz2[:, 4:6], tc2[:], ALU.mult)

                nc.sync.dma_start(out=d_yout[:], in_=youts[:])
                nc.sync.dma_start(out=d_Yout[:], in_=Youts[:])

    nc.compile()
    return nc


_NC = None


def kernel(data, length, lstm_params, params):
    global _NC
    shared = pack_weights(lstm_params, params)
    per_core = pack_percore(shared, data, length, T_FULL)
    if _NC is None:
        _NC = build_nc()
    res = run_bass_kernel_spmd(_NC, per_core, list(range(B)))
    yout = np.stack([_f32(res.results[b]["yout"])[0] for b in range(B)])
    Yout = np.stack([_f32(res.results[b]["Yout"])[0] for b in range(B)])
    return (yout, Yout)
